# revision 2
# baseline (speedup 1.0000x reference)
"""Trainium2 Bass kernel for BasePanHead.lay_masks (panoptic mask laying).

Self-contained: builds and runs an 8-core SPMD Bass kernel via PJRT (axon).

Algorithm: instances sorted by score descending; sequential scan with a
global keep decision per instance:
    u = |mask & uncovered| ; a = |mask| ; keep <=> a >= 1 and a - u <= 0.5*(a+1e-5)
which is exactly equivalent (for integer a, u) to  2*(a+u) >= 3*a.
Kept instances lay their newly-covered pixels with their (1-based) kept rank.

Device mapping: pixels are sharded across 8 NeuronCores (133,300 px each,
padded to 128x1042). Each step, every core computes a fused
multiply-accumulate  q = sum(2*mask*c)  (c in {1=covered,2=uncovered}) on the
vector engine and  -3*area  on the scalar engine, broadcasts the two partial
sums to all cores over direct core-to-core SDMA (remote_dma_broadcast), and
every core redundantly reduces them (PE matvec + vector reduce) to the single
integer m = 2*sum(q) - 3*sum(a), makes the identical keep decision with a
sequencer register compare, and (only when kept) updates its coverage map and
id map.  The id map stores the step number; the host remaps step numbers to
kept ranks at the end.
"""
import numpy as np

import concourse.bass as bass
import concourse.bacc as bacc
from concourse import mybir, library_config

AO = mybir.AluOpType
DT = mybir.dt
P = 128
F = 1042
NCORES = 8
N_INST = 100
H_FULL, W_FULL = 800, 1333


def _build_kernel(nc, masks_in, idm_out, n_steps, prep_window=4,
                  mask_count=None):
    N = n_steps
    MC = mask_count if mask_count is not None else n_steps
    W = prep_window
    from contextlib import ExitStack
    with ExitStack() as ES:
        def sb(name, shape, dtype):
            return ES.enter_context(nc.sbuf_tensor(name, shape, dtype))
        def ps(name, shape, dtype):
            return ES.enter_context(nc.psum_tensor(name, shape, dtype))
        def sem(name):
            return ES.enter_context(nc.semaphore(name=name))
        mb = [sb(f"mb{i}", [P, F], DT.int8) for i in range(4)]
        cc = sb("cc", [P, F], DT.int8)
        tt_ = sb("tt", [P, F], DT.int8)
        ind = sb("ind", [P, F], DT.int8)
        ascr = [sb(f"ascr{i}", [P, F], DT.int8) for i in range(2)]
        idm = sb("idm", [P, F], DT.int8)
        ones = sb("ones", [P, 1], DT.float32)
        slot = sb("slot", [128, 4], DT.float32)
        arr = sb("arr", [128, 32], DT.float32)
        totv = sb("totv", [1, 2], DT.int32)
        psum_r = [ps(f"psum_r{i}", [1, 16], DT.float32) for i in range(2)]
        dve_sem = sem("dve_sem")
        bsem = sem("bsem")
        act_sem = sem("act_sem")
        pe_sem = sem("pe_sem")
        osem = sem("osem")
        prep_sem = sem("prep_sem")
        dsems = [sem(f"dsem{i}") for i in range(4)]
        lsems = [sem(f"lsem{p}") for p in range(2)]
        rsems = [[sem(f"rs{i}p{p}") for p in range(2)] for i in range(8)]
        block = ES.enter_context(nc.Block())

        PRO = 3  # c, idm, ones memsets

        def dve_idx(k, j):  # j in {0: stt#1, 1: tot}
            return PRO + 2 * k + j

        @block.sync
        def _(sync):
            for k in range(N):
                if k >= 4:
                    sync.wait_ge(dve_sem, dve_idx(k - 4, 0) + 1)
                    sync.wait_ge(act_sem, k - 3)
                sync.dma_start(mb[k % 4][:, :], masks_in[k % MC, :, :]).then_inc(
                    dsems[k % 4], 16)
            sync.wait_ge(bsem, 3 * N)
            sync.dma_start(idm_out[:, :], idm[:, :]).then_inc(osem, 16)

        @block.vector
        def _(v):
            v.memset(cc[:, :], 2).then_inc(dve_sem, 1)
            v.memset(idm[:, :], 0).then_inc(dve_sem, 1)
            v.memset(ones[:, :], 1.0).then_inc(dve_sem, 1)
            q = v.alloc_register("qreg")
            for k in range(N):
                ph = k % 2
                # --- stt#1: tt = 2*mask*cc, accum -> slot q-col ---
                if k > 0:
                    v.wait_ge(bsem, 3 * k)
                else:
                    v.wait_ge(dve_sem, 3)
                v.wait_ge(dsems[k % 4], 16 * (k // 4 + 1))
                if k >= 2:
                    v.wait_ge(lsems[ph], 16 * (k // 2))
                v.scalar_tensor_tensor(
                    tt_[:, :], mb[k % 4][:, :], 2.0, cc[:, :],
                    op0=AO.mult, op1=AO.mult,
                    accum_out=slot[:, 2 * ph : 2 * ph + 1],
                ).then_inc(dve_sem, 1)
                # --- m = sum over all 16 partials (2q_j and -3a_j) ---
                v.wait_ge(pe_sem, k + 1)
                with nc.allow_low_precision(reason="exact int sums"):
                    v.tensor_reduce(totv[0:1, 0:1], psum_r[ph][0:1, :],
                                    axis=mybir.AxisListType.X,
                                    op=AO.add).then_inc(dve_sem, 1)
                v.wait_ge(dve_sem, dve_idx(k, 1) + 1)
                v.reg_load(q, totv[0:1, 0:1])
                with v.If_cmp(q, 0, "IS_GE"):
                    # keep: ind = (tt == 4); cc -= ind; idm = max(idm, (k+1)*ind)
                    v.tensor_scalar(ind[:, :], tt_[:, :], 4.0, None,
                                    op0=AO.is_equal).then_inc(bsem, 1)
                    v.wait_ge(bsem, 3 * k + 1)
                    v.tensor_tensor(cc[:, :], cc[:, :], ind[:, :],
                                    AO.subtract).then_inc(bsem, 1)
                    v.scalar_tensor_tensor(
                        idm[:, :], ind[:, :], float(k + 1), idm[:, :],
                        op0=AO.mult, op1=AO.max).then_inc(bsem, 1)
                with v.Else():
                    v.engine_nop().then_inc(bsem, 3)

        @block.scalar
        def _(sc):
            sc.wait_ge(dsems[0], 16)
            sc.activation(ascr[0][:, :], mb[0][:, :],
                          mybir.ActivationFunctionType.Copy, scale=-3.0,
                          accum_out=slot[:, 1:2]).then_inc(act_sem, 1)
            for k in range(N - 1):
                ph = k % 2
                sc.wait_ge(dsems[(k + 1) % 4], 16 * ((k + 1) // 4 + 1))
                if k >= 1:
                    sc.wait_ge(act_sem, k)
                    sc.wait_ge(lsems[1 - ph], 16 * ((k + 1) // 2))
                sc.activation(ascr[(k + 1) % 2][:, :], mb[(k + 1) % 4][:, :],
                              mybir.ActivationFunctionType.Copy, scale=-3.0,
                              accum_out=slot[:, 2 * (1 - ph) + 1 :
                                             2 * (1 - ph) + 2]).then_inc(act_sem, 1)

        @block.tensor
        def _(te):
            for k in range(N):
                ph = k % 2
                for j in range(8):
                    te.wait_ge(rsems[j][ph], 2 * (k // 2 + 1))
                if k >= 2:
                    te.wait_ge(dve_sem, dve_idx(k - 2, 1) + 1)
                else:
                    te.wait_ge(dve_sem, 3)
                te.matmul(psum_r[ph][:, :], ones[:, 0:1],
                          arr[:, 16 * ph : 16 * ph + 16],
                          start=True, stop=True).then_inc(pe_sem, 1)

        @block.gpsimd
        def _(g):
            g.load_library(library_config.remote_dma)
            pid_reg = g.to_reg(g.partition_id())
            for i in range(8):
                with g.If_eq(pid_reg, i):
                    for k in range(min(W, N)):
                        ph = k % 2
                        g.remote_dma_broadcast(
                            out_ap=arr[:, 16 * ph + 2 * i : 16 * ph + 2 * i + 2],
                            in_ap=slot[:, 2 * ph : 2 * ph + 2],
                            remote_sem=rsems[i][ph],
                            local_sem=lsems[ph],
                            rdests=[(0, d) for d in range(8)],
                        ).then_inc(prep_sem, 1)
                    for k in range(N):
                        if k + W < N:
                            ph2 = (k + W) % 2
                            g.remote_dma_broadcast(
                                out_ap=arr[:, 16 * ph2 + 2 * i : 16 * ph2 + 2 * i + 2],
                                in_ap=slot[:, 2 * ph2 : 2 * ph2 + 2],
                                remote_sem=rsems[i][ph2],
                                local_sem=lsems[ph2],
                                rdests=[(0, d) for d in range(8)],
                            ).then_inc(prep_sem, 1)
                        g.wait_ge(prep_sem, k + 1)
                        g.wait_ge(dve_sem, dve_idx(k, 0) + 1)
                        g.wait_ge(act_sem, k + 1)
                        g.trigger_dma(1)
    return nc


class _Runner:
    """Cached multi-core PJRT runner for a prebuilt Bass module."""

    def __init__(self, nc, n_cores):
        import jax
        from jax.sharding import Mesh, PartitionSpec
        from jax.experimental.shard_map import shard_map
        from concourse.bass2jax import (
            _bass_exec_p, install_neuronx_cc_hook, partition_id_tensor)
        install_neuronx_cc_hook()
        self.jax = jax
        self.n_cores = n_cores
        partition_name = (
            nc.partition_id_tensor.name if nc.partition_id_tensor else None)
        in_names, out_names, out_avals, zero_outs = [], [], [], []
        for alloc in nc.m.functions[0].allocations:
            if not isinstance(alloc, mybir.MemoryLocationSet):
                continue
            name = alloc.memorylocations[0].name
            if alloc.kind == "ExternalInput":
                if name != partition_name:
                    in_names.append(name)
            elif alloc.kind == "ExternalOutput":
                out_names.append(name)
                shape = tuple(alloc.tensor_shape)
                dtype = mybir.dt.np(alloc.dtype)
                out_avals.append(jax.core.ShapedArray(shape, dtype))
                zero_outs.append(np.zeros(shape, dtype))
        self.in_names, self.out_names = in_names, out_names
        self.out_avals, self.zero_outs = out_avals, zero_outs
        n_params, n_outs = len(in_names), len(out_names)
        all_in_names = list(in_names) + list(out_names)
        if partition_name is not None:
            all_in_names.append(partition_name)

        def _body(*args):
            operands = list(args)
            if partition_name is not None:
                operands.append(partition_id_tensor())
            outs = _bass_exec_p.bind(
                *operands,
                out_avals=tuple(out_avals),
                in_names=tuple(all_in_names),
                out_names=tuple(out_names),
                lowering_input_output_aliases=(),
                sim_require_finite=True,
                sim_require_nnan=True,
                nc=nc,
            )
            return tuple(outs)

        devices = jax.devices()[:n_cores]
        assert len(devices) == n_cores, (
            f"need {n_cores} neuron cores, found {len(jax.devices())}")
        self.mesh = Mesh(np.asarray(devices), ("core",))
        in_specs = (PartitionSpec("core"),) * (n_params + n_outs)
        out_specs = (PartitionSpec("core"),) * n_outs
        self.fn = jax.jit(
            shard_map(_body, mesh=self.mesh, in_specs=in_specs,
                      out_specs=out_specs, check_rep=False),
            keep_unused=True)
        self._spec = jax.sharding.NamedSharding(self.mesh, PartitionSpec("core"))

    def stage(self, in_maps):
        n = self.n_cores
        per_core = [[np.asarray(m[name]) for name in self.in_names]
                    for m in in_maps]
        concat_in = [
            np.concatenate([per_core[c][i] for c in range(n)], axis=0)
            for i in range(len(self.in_names))]
        concat_zero = [
            np.zeros((n * z.shape[0], *z.shape[1:]), z.dtype)
            for z in self.zero_outs]
        self._dev_args = [self.jax.device_put(a, self._spec)
                          for a in concat_in + concat_zero]

    def run(self):
        outs = self.fn(*self._dev_args)
        self.jax.block_until_ready(outs)
        n = self.n_cores
        return [
            {name: np.asarray(outs[i]).reshape(n, *self.out_avals[i].shape)[c]
             for i, name in enumerate(self.out_names)}
            for c in range(n)]


_CACHED = {}


def _get_runner():
    if "r" not in _CACHED:
        nc = bacc.Bacc("TRN2", target_bir_lowering=False, debug=False,
                       enable_asserts=False, num_devices=NCORES)
        masks_in = nc.dram_tensor(
            "masks_in", [N_INST, P, F], mybir.dt.int8,
            kind="ExternalInput").ap()
        idm_out = nc.dram_tensor(
            "idm_out", [P, F], mybir.dt.int8, kind="ExternalOutput").ap()
        _build_kernel(nc, masks_in, idm_out, N_INST)
        nc.compile()
        _CACHED["r"] = _Runner(nc, NCORES)
    return _CACHED["r"]


def _shard_masks(masks_sorted_u8):
    n = masks_sorted_u8.shape[0]
    flat = masks_sorted_u8.reshape(n, -1)
    per = flat.shape[1] // NCORES
    outs = []
    for i in range(NCORES):
        sl = flat[:, i * per : (i + 1) * per]
        sl = np.pad(sl, ((0, 0), (0, P * F - per)))
        outs.append(np.ascontiguousarray(sl.reshape(n, P, F).astype(np.int8)))
    return outs


def kernel(bboxes, labels, segm_masks):
    bboxes = np.asarray(bboxes)
    labels = np.asarray(labels)
    segm_masks = np.asarray(segm_masks)
    n, H, Wd = segm_masks.shape
    assert (n, H, Wd) == (N_INST, H_FULL, W_FULL), "kernel is shape-specialized"

    order = np.argsort(-bboxes[:, -1], kind="stable")
    masks_s = segm_masks[order].astype(np.uint8)
    labels_s = labels[order]

    r = _get_runner()
    r.stage([{"masks_in": s} for s in _shard_masks(masks_s)])
    outs = r.run()

    # reassemble: idm slices hold step numbers (1-based) of kept instances
    per = (H * Wd) // NCORES
    flat = np.concatenate(
        [np.asarray(o["idm_out"], dtype=np.uint8).reshape(-1)[:per]
         for o in outs])
    kept_steps = np.unique(flat)
    kept_steps = kept_steps[kept_steps > 0]
    keeps = np.zeros(n, bool)
    keeps[kept_steps - 1] = True
    remap = np.zeros(n + 1, np.int32)
    remap[kept_steps] = np.arange(1, len(kept_steps) + 1, dtype=np.int32)
    id_map = remap[flat].reshape(H, Wd)

    perm = np.argsort(~keeps, kind="stable")
    inst_labels = np.where(keeps[perm], labels_s[perm], -1).astype(np.int32)
    return id_map, inst_labels
